# revision 33
# baseline (speedup 1.0000x reference)
"""Trainium2 Bass kernel for nn_DualAttention (sparse_attention).

Algorithm notes
---------------
The reference gathers per-pair mention blocks hfo/tfo = mention_embed[b, h/t]
([N,16,768]) and projects them per pair. But the projections depend only on
the (batch, entity) index, of which there are just B*E = 168, so we compute
relu(X @ W.T) per *entity* (24x less matmul work), then combine per pair:

  s[n,i,j] = hc[eh][i] + qv[et][j] + tq[et][i] * hf16[eh][i,j]   (+ masks)
  h_weight = softmax_i(max_j s);  start_re = h_weight @ hf[eh]
  t_weight = softmax_j(max_i s);  end_re   = t_weight @ tf[et]

Distribution over 8 cores: pairs are sorted by head entity (and separately by
tail entity); core k takes sorted block k of 512 pairs, so its pairs touch a
narrow contiguous band of entities. Each core projects only its band rows
(host passes the pre-transposed mention rows for the band), computes the
small per-entity tables (hc, hf16 / qv, tq) for its band, and an AllGather
shares those tables (tiny) with all cores. Per-pair gathers from the tables
are one-hot matmuls; the final weighted sums are banded matmuls
G.T @ hf_band where G is built on-device from the softmax weights and a
host-provided band mask. The entity_embed half of the outputs is a pure
input gather and is assembled on the host.

Matmuls run in float32r (full-rate fp32 with reduced mantissa, ~1e-4 rel
err); set DTYPE_MM = "f32" for exact-rate-limited fp32.
"""

import numpy as np

import concourse.bass as bass
import concourse.mybir as mybir
import concourse.tile as tile
from concourse.bass_utils import run_bass_kernel_spmd

# problem constants
H = 768
B, E, M = 4, 42, 16
NENT = B * E            # 168
N = 4096
NC = 8
PPC = N // NC           # 512 pairs per core
MT = PPC // 128         # 4 m-tiles of pairs per core
KT = H // 128           # 6 k-tiles over hidden dim
NEG = -1e9

F32 = mybir.dt.float32
F32R = mybir.dt.float32r
DTYPE_MM = F32R         # matmul dtype for the heavy matmuls


def _split_multi_waits(nc, max_waits=1):
    """walrus codegen in this container rejects >1 sync wait per instruction.

    Move extra waits onto pure-wait EventSemaphore instructions inserted just
    before, on the same engine (engine queues are serial, so ordering and
    semantics are preserved)."""
    for fn in nc.m.functions:
        for bb in fn.blocks:
            new = []
            changed = False
            for ins in bb.instructions:
                si = ins.sync_info
                if si is not None and si.on_wait and len(si.on_wait) > max_waits:
                    waits = list(si.on_wait)
                    for i, w in enumerate(waits[:-max_waits]):
                        ev = mybir.InstEventSemaphore(
                            name=f"{ins.name}-xw{i}", engine=ins.engine
                        )
                        ev.sync_info = mybir.SyncInfo(on_wait=[w], on_update=[])
                        ev.debug = ins.debug
                        new.append(ev)
                    si.on_wait = waits[-max_waits:]
                    changed = True
                new.append(ins)
            if changed:
                bb.instructions = new


def _band(ent_sorted):
    """Aligned 128-row band of mention rows covering the given entities."""
    lo_row = 16 * int(ent_sorted.min())
    hi_row = 16 * int(ent_sorted.max()) + 16
    lo_tile = lo_row // 128
    nb = (hi_row - 128 * lo_tile + 127) // 128
    return lo_tile, nb


def _prep(inputs):
    """Host-side sharding: indices, sort orders, bands, one-hots, masks."""
    f32 = np.float32
    mention = np.ascontiguousarray(inputs["mention_embed"], dtype=f32)
    mention_flat = mention.reshape(NENT * M, H)          # row 16*e + i
    b_ind = np.asarray(inputs["b_ind"]).astype(np.int64)
    h_ind = np.asarray(inputs["h_ind"]).astype(np.int64)
    t_ind = np.asarray(inputs["t_ind"]).astype(np.int64)
    mention_num = np.asarray(inputs["mention_num"]).astype(np.int64)

    eh = (b_ind * E + h_ind).astype(np.int64)
    et = (b_ind * E + t_ind).astype(np.int64)
    mnum_flat = mention_num.reshape(NENT)

    h_order = np.argsort(eh, kind="stable")
    t_order = np.argsort(et, kind="stable")

    lo_h, nb_h, lo_t, nb_t = [], [], [], []
    for k in range(NC):
        lo, nb = _band(eh[h_order[k * PPC:(k + 1) * PPC]])
        lo_h.append(lo); nb_h.append(nb)
        lo, nb = _band(et[t_order[k * PPC:(k + 1) * PPC]])
        lo_t.append(lo); nb_t.append(nb)
    NBH = max(nb_h)
    NBT = max(nb_t)

    # table slot count per core: power-of-two-ish divisor of 128
    def slots_for(nb):
        need = nb * 8
        for s in (16, 32, 64, 128):
            if need <= s:
                return s
        raise ValueError(f"band too wide: {nb} tiles")
    SLH = slots_for(NBH)
    SLT = slots_for(NBT)
    TH = NC * SLH // 128        # Htable k-tiles
    TT = NC * SLT // 128

    # owner core + table row for each entity (first band containing it)
    def table_rows(lo_list, nb, slots):
        rows = np.full(NENT, -1, np.int64)
        for k in reversed(range(NC)):
            base_ent = 8 * lo_list[k]
            ents = np.arange(base_ent, min(base_ent + nb * 8, NENT))
            rows[ents] = k * slots + (ents - base_ent)
        return rows
    hrow = table_rows(lo_h, NBH, SLH)
    trow = table_rows(lo_t, NBT, SLT)

    iota16 = np.tile(np.arange(16, dtype=f32)[None, :], (128, 1))

    per_core = []
    for k in range(NC):
        ph = h_order[k * PPC:(k + 1) * PPC]
        pt = t_order[k * PPC:(k + 1) * PPC]
        ehh, eth = eh[ph], et[ph]      # entity ids for h-ordered pairs
        eht, ett = eh[pt], et[pt]      # ... for t-ordered pairs

        # band mention rows, transposed, zero-padded
        def xt_for(lo, nb):
            rows = np.zeros((nb * 128, H), f32)
            g0 = 128 * lo
            g1 = min(g0 + nb * 128, NENT * M)
            rows[: g1 - g0] = mention_flat[g0:g1]
            return np.ascontiguousarray(rows.T)          # [768, nb*128]
        xt_h = xt_for(lo_h[k], NBH)
        xt_t = xt_for(lo_t[k], NBT)

        # value vectors for device-built one-hots and band masks:
        # [hrow(ehh) | trow(eth) | hrow(eht) | trow(ett) | eh(h-order) | et(t-order)]
        vals = np.concatenate([
            hrow[ehh], trow[eth], hrow[eht], trow[ett], ehh, ett,
        ]).astype(f32)[None, :]                           # [1, 6*512]

        # entcols[p, c]: global entity id of band row 128*c + p (head cols
        # first, then tail cols)
        pp = np.arange(128)
        entc = [((128 * lo_h[k] + 128 * c + pp) // 16).astype(f32)
                for c in range(NBH)]
        entc += [((128 * lo_t[k] + 128 * c + pp) // 16).astype(f32)
                 for c in range(NBT)]
        entcols = np.stack(entc, axis=1)                  # [128, NBH+NBT]

        # per-pair mention counts as [128, MT] per-partition scalars
        def nums(vals):
            return np.ascontiguousarray(
                vals.astype(f32).reshape(MT, 128).T)     # [128, MT]
        numcols = np.concatenate(
            [nums(mnum_flat[ehh]), nums(mnum_flat[eth]),
             nums(mnum_flat[eht]), nums(mnum_flat[ett])], axis=1)  # [128,16]

        per_core.append(dict(
            xt_h=xt_h, xt_t=xt_t, vals=vals, entcols=entcols,
            nums=numcols,
        ))

    wts = np.ascontiguousarray(np.concatenate(
        [np.asarray(inputs["W_head"], dtype=f32).T,
         np.asarray(inputs["W_tail"], dtype=f32).T], axis=1))     # [768, 1536]
    wvec = np.concatenate([
        np.tile(np.asarray(inputs["w_c"], f32)[None, :], (128, 1)),
        np.tile(np.asarray(inputs["w_q"], f32)[None, :], (128, 1)),
        np.tile(np.asarray(inputs["w_cq"], f32)[None, :], (128, 1)),
    ], axis=1)                                                    # [128, 2304]

    meta = dict(NBH=NBH, NBT=NBT, SLH=SLH, SLT=SLT, TH=TH, TT=TT)
    TMX = max(TH, TT)
    iotaP = np.stack([(128 * c + np.arange(128)).astype(f32)
                      for c in range(TMX)], axis=1)       # [128, TMX]
    repm = (np.arange(16)[:, None] == (np.arange(128) % 16)[None, :]).astype(f32)
    shared = dict(wts=wts, wvec=wvec, iota16=iota16, iotaP=iotaP,
                  identity=np.eye(128, dtype=f32), repm=repm)
    post = dict(h_order=h_order, t_order=t_order, eh=eh, et=et)
    return meta, shared, per_core, post


def _build(meta, sim_single=False):
    NBH, NBT = meta["NBH"], meta["NBT"]
    SLH, SLT = meta["SLH"], meta["SLT"]
    TH, TT = meta["TH"], meta["TT"]
    # two allgather shards: head = [hc | hf16], tail = [qv | tq] — split so
    # the head collective can start while tail projections still run
    sec_hc = 0
    sec_hf16 = SLH * 16
    SHARD_H = SLH * 16 * 17
    sec_qv = 0
    sec_tq = SLT * 16
    SHARD_T = SLT * 32

    nc = bass.Bass("TRN2", num_devices=(1 if sim_single else NC))
    xt_h = nc.dram_tensor("xt_h", [H, NBH * 128], F32R, kind="ExternalInput")
    xt_t = nc.dram_tensor("xt_t", [H, NBT * 128], F32R, kind="ExternalInput")
    wts = nc.dram_tensor("wts", [H, 2 * H], F32R, kind="ExternalInput")
    wvec = nc.dram_tensor("wvec", [128, 3 * H], F32, kind="ExternalInput")
    iota16 = nc.dram_tensor("iota16", [128, 16], F32, kind="ExternalInput")
    TMX = max(TH, TT)
    vals = nc.dram_tensor("vals", [1, 6 * PPC], F32, kind="ExternalInput")
    entcols = nc.dram_tensor("entcols", [128, NBH + NBT], F32, kind="ExternalInput")
    iotaP = nc.dram_tensor("iotaP", [128, TMX], F32, kind="ExternalInput")
    nums = nc.dram_tensor("nums", [128, 4 * MT], F32, kind="ExternalInput")
    identity = nc.dram_tensor("identity", [128, 128], F32, kind="ExternalInput")
    repm = nc.dram_tensor("repm", [16, 128], F32, kind="ExternalInput")
    reh = nc.dram_tensor("reh", [PPC, H], F32, kind="ExternalOutput")
    ret = nc.dram_tensor("ret", [PPC, H], F32, kind="ExternalOutput")

    with tile.TileContext(nc, num_cores=NC) as tc:
        with (
            tc.tile_pool(name="const", bufs=1) as cpool,
            tc.tile_pool(name="band", bufs=1) as bpool,
            tc.tile_pool(name="work", bufs=3) as wpool,
            tc.tile_pool(name="keep", bufs=1) as gpool,
            tc.tile_pool(name="small", bufs=4) as spool,
            # "proj" big psum: [128,768] = 2 banks x 2 bufs; "sm" small psum:
            # 1 bank x 4 bufs -> 8 banks total, exactly PSUM capacity
            tc.tile_pool(name="psum", bufs=2, space="PSUM") as ppool,
            tc.tile_pool(name="psg", bufs=4, space="PSUM") as pgpool,
            tc.tile_pool(name="dram", bufs=1, space="DRAM") as dpool,
        ):
            # ---- big matmul operands first, interleaved by k-tile, so the
            # first projections can start as soon as their slices land ----
            wt_r, xth, xtt = [], [], []
            for kt in range(KT):
                w = cpool.tile([128, 2 * H], DTYPE_MM, tag=f"wt{kt}")
                nc.sync.dma_start(w[:, H:], wts.ap()[kt * 128:(kt + 1) * 128, H:])
                tt = bpool.tile([128, NBT * 128], DTYPE_MM, tag=f"xtt{kt}")
                nc.sync.dma_start(tt[:], xt_t.ap()[kt * 128:(kt + 1) * 128, :])
                wt_r.append(w)
                xtt.append(tt)
            for kt in range(KT):
                nc.sync.dma_start(
                    wt_r[kt][:, :H], wts.ap()[kt * 128:(kt + 1) * 128, :H])
                th = bpool.tile([128, NBH * 128], DTYPE_MM, tag=f"xth{kt}")
                nc.sync.dma_start(th[:], xt_h.ap()[kt * 128:(kt + 1) * 128, :])
                xth.append(th)
            wvec_sb = cpool.tile([128, 3 * H], F32)
            nc.sync.dma_start(wvec_sb[:], wvec.ap())
            iota_sb = cpool.tile([128, 16], F32)
            nc.sync.dma_start(iota_sb[:], iota16.ap())
            nums_sb = cpool.tile([128, 4 * MT], F32)
            nc.sync.dma_start(nums_sb[:], nums.ap())
            entcols_sb = cpool.tile([128, NBH + NBT], F32)
            nc.sync.dma_start(entcols_sb[:], entcols.ap())
            iotaP_sb = cpool.tile([128, TMX], F32)
            nc.sync.dma_start(iotaP_sb[:], iotaP.ap())
            vrep = cpool.tile([128, 6 * PPC], F32)
            nc.gpsimd.dma_start(
                vrep[:],
                bass.AP(tensor=vals.ap().tensor, offset=0,
                        ap=[[0, 128], [1, 6 * PPC]]))
            ident = cpool.tile([128, 128], F32)
            nc.gpsimd.dma_start(ident[:], identity.ap())
            repm_sb = cpool.tile([16, 128], F32R)
            nc.gpsimd.dma_start(repm_sb[:], repm.ap())

            # ---- phase A: band projections hf = relu(X @ W_head.T) ----
            def project(xt_tiles, nb, woff, tag):
                out = []
                for mt in range(nb):
                    ps = ppool.tile([128, H], F32, space="PSUM", tag="proj")
                    for half in range(2):
                        sl = slice(woff + half * 512, woff + min(768, (half + 1) * 512))
                        for kt in range(KT):
                            nc.tensor.matmul(
                                ps[:, half * 512: half * 512 + (sl.stop - sl.start)],
                                lhsT=xt_tiles[kt][:, mt * 128:(mt + 1) * 128],
                                rhs=wt_r[kt][:, sl],
                                start=(kt == 0), stop=(kt == KT - 1),
                            )
                    t = bpool.tile([128, H], DTYPE_MM, tag=f"{tag}{mt}")
                    if mt % 2 == 0:
                        nc.scalar.activation(t[:], ps[:],
                                             mybir.ActivationFunctionType.Relu)
                    else:
                        nc.vector.tensor_scalar_max(t[:], ps[:], 0.0)
                    out.append(t)
                return out
            tfb = project(xtt, NBT, H, "tfb")
            hfb = project(xth, NBH, 0, "hfb")

            # ---- phase A2: per-entity smalls + allgather ----
            stage_h = dpool.tile([SHARD_H], F32R)
            stage_t = dpool.tile([SHARD_T], F32R)
            cc_h = dpool.tile(
                [NC * SHARD_H], F32R,
                **({} if sim_single else {"addr_space": "Shared"}))
            cc_t = dpool.tile(
                [NC * SHARD_T], F32R,
                **({} if sim_single else {"addr_space": "Shared"}))
            assert SHARD_H % 128 == 0 and SHARD_T % 128 == 0
            zero_sb = cpool.tile([128, SHARD_H // 128], F32)
            nc.vector.memset(zero_sb, 0.0)
            nc.sync.dma_start(
                stage_h[:].rearrange("(p c) -> p c", p=128),
                zero_sb[:, : SHARD_H // 128].bitcast(F32R),
            )
            nc.sync.dma_start(
                stage_t[:].rearrange("(p c) -> p c", p=128),
                zero_sb[:, : SHARD_T // 128].bitcast(F32R),
            )

            def rowdot(src_tile, wcol, acc_ap, eng=None):
                prod = wpool.tile([128, H], F32, tag="prod")
                (eng or nc.vector).tensor_tensor(
                    out=prod[:], in0=src_tile[:].bitcast(F32),
                    in1=wvec_sb[:, wcol * H:(wcol + 1) * H],
                    op=mybir.AluOpType.mult)
                with nc.allow_low_precision(reason="f32r stage values"):
                    nc.scalar.activation(
                        prod[:], prod[:], mybir.ActivationFunctionType.Copy,
                        accum_out=acc_ap)

            def acc_dma(stage, sec, acc_tile, nb):
                dst = bass.AP(tensor=stage[:].tensor, offset=sec,
                              ap=[[1, 128], [128, nb]])
                nc.sync.dma_start(dst, acc_tile[:, :nb])

            acc_hc = spool.tile([128, NBH], F32R, tag="acc_hc")
            acc_qv = spool.tile([128, NBT], F32R, tag="acc_qv")
            acc_tq = spool.tile([128, NBT], F32R, tag="acc_tq")
            def do_collective(stage, cc):
                if sim_single:
                    sz = stage.shape[0]
                    nc.sync.dma_start(cc[0:sz], stage[:])
                    nc.sync.dma_start(cc[(NC - 1) * sz: NC * sz], stage[:])
                else:
                    nc.gpsimd.collective_compute(
                        "AllGather", mybir.AluOpType.bypass,
                        replica_groups=[list(range(NC))],
                        ins=[stage.opt()], outs=[cc.opt()],
                    )

            for mt in range(NBT):
                rowdot(tfb[mt], 1, acc_qv[:, mt:mt + 1])
                rowdot(tfb[mt], 2, acc_tq[:, mt:mt + 1])
            acc_dma(stage_t, sec_qv, acc_qv, NBT)
            acc_dma(stage_t, sec_tq, acc_tq, NBT)
            do_collective(stage_t, cc_t)

            for mt in range(NBH):
                rowdot(hfb[mt], 0, acc_hc[:, mt:mt + 1])
                nc.sync.dma_start(
                    stage_h[sec_hf16 + mt * 2048: sec_hf16 + (mt + 1) * 2048]
                    .rearrange("(p c) -> p c", p=128),
                    hfb[mt][:, :16],
                )
            acc_dma(stage_h, sec_hc, acc_hc, NBH)
            do_collective(stage_h, cc_h)


            # ---- build gathered tables in SBUF ----
            def tbl_src(cc, shard, sec, per_slot, tt, slots, width):
                cores_per_tile = 128 // slots
                return bass.AP(
                    tensor=cc[:].tensor,
                    offset=tt * cores_per_tile * shard + sec,
                    ap=[[shard, cores_per_tile], [per_slot, slots], [1, width]],
                )
            Htab, Ttab = [], []
            for tt in range(TH):
                t = cpool.tile([128, 272], DTYPE_MM, tag=f"htab{tt}")
                nc.gpsimd.dma_start(
                    t[:, 0:16], tbl_src(cc_h, SHARD_H, sec_hc, 16, tt, SLH, 16))
                nc.gpsimd.dma_start(
                    t[:, 16:272], tbl_src(cc_h, SHARD_H, sec_hf16, 256, tt, SLH, 256))
                Htab.append(t)
            for tt in range(TT):
                t = cpool.tile([128, 32], DTYPE_MM, tag=f"ttab{tt}")
                nc.sync.dma_start(
                    t[:, 0:16], tbl_src(cc_t, SHARD_T, sec_qv, 16, tt, SLT, 16))
                nc.sync.dma_start(
                    t[:, 16:32], tbl_src(cc_t, SHARD_T, sec_tq, 16, tt, SLT, 16))
                Ttab.append(t)

            # ---- device-built one-hots: oh[p, n] = (rowvals[n] == 128*kt+p)
            def build_oh(vcol, ktiles, tag):
                tiles = []
                for kt in range(ktiles):
                    t = bpool.tile([128, PPC], DTYPE_MM, tag=f"{tag}{kt}")
                    nc.gpsimd.tensor_scalar(
                        out=t[:],
                        in0=vrep[:, vcol * PPC:(vcol + 1) * PPC],
                        scalar1=iotaP_sb[:, kt:kt + 1], scalar2=None,
                        op0=mybir.AluOpType.is_equal)
                    tiles.append(t)
                return tiles
            ohHh = build_oh(0, TH, "ohHh")
            ohTh = build_oh(1, TT, "ohTh")
            ohHt = build_oh(2, TH, "ohHt")
            ohTt = build_oh(3, TT, "ohTt")

            # ---- precomputed additive masks (only need nums+iota, so these
            # run at t~0 in the DMA shadow): m[col][p,i] = (i>=num)*NEG ----
            masks = []
            for col in range(4):
                for mt in range(MT):
                    mk = gpool.tile([128, 16], F32, tag=f"msk{col}_{mt}")
                    nc.vector.tensor_scalar(
                        out=mk[:], in0=iota_sb[:],
                        scalar1=nums_sb[:, col * MT + mt: col * MT + mt + 1],
                        scalar2=NEG,
                        op0=mybir.AluOpType.is_ge, op1=mybir.AluOpType.mult)
                    masks.append(mk)

            # ---- phase B: per-pair scores + softmax weights ----
            def phase_b(ohH, ohT, hn_col, tn_col, reduce_axis):
                """Returns list of MT [128,16] weight tiles."""
                weights = []
                for mt in range(MT):
                    gps = pgpool.tile([128, 272], F32, space="PSUM", tag="sm")
                    for kt in range(TH):
                        nc.tensor.matmul(
                            gps[:], lhsT=ohH[kt][:, mt * 128:(mt + 1) * 128],
                            rhs=Htab[kt][:], start=(kt == 0), stop=(kt == TH - 1))
                    Hg = gps

                    tps = pgpool.tile([128, 32], F32, space="PSUM", tag="sm")
                    for kt in range(TT):
                        nc.tensor.matmul(
                            tps[:], lhsT=ohT[kt][:, mt * 128:(mt + 1) * 128],
                            rhs=Ttab[kt][:], start=(kt == 0), stop=(kt == TT - 1))
                    Tg = wpool.tile([128, 32], F32, tag="Tg")
                    nc.vector.tensor_copy(Tg[:], tps[:])

                    # s[p,i,j] = (hc[i]+maski[i]) + (qv[j]+maskj[j]) + tq[i]*F[i,j]
                    mi = masks[hn_col * MT + mt]
                    mj = masks[tn_col * MT + mt]
                    a2 = spool.tile([128, 16], F32, tag="a2")
                    nc.vector.tensor_tensor(out=a2[:], in0=Hg[:, 0:16],
                                            in1=mi[:], op=mybir.AluOpType.add)
                    b2 = spool.tile([128, 16], F32, tag="b2")
                    nc.vector.tensor_tensor(out=b2[:], in0=Tg[:, 0:16],
                                            in1=mj[:], op=mybir.AluOpType.add)
                    s = wpool.tile([128, 16, 16], F32, tag="s")
                    nc.vector.tensor_tensor(
                        out=s[:],
                        in0=Tg[:, 16:32, None].to_broadcast((128, 16, 16)),
                        in1=Hg[:, 16:272].rearrange("p (i j) -> p i j", i=16),
                        op=mybir.AluOpType.mult)
                    u = wpool.tile([128, 16, 16], F32, tag="u")
                    nc.gpsimd.tensor_tensor(
                        out=u[:],
                        in0=a2[:, :, None].to_broadcast((128, 16, 16)),
                        in1=b2[:, None, 0:16].to_broadcast((128, 16, 16)),
                        op=mybir.AluOpType.add)
                    nc.vector.tensor_tensor(out=s[:], in0=s[:], in1=u[:],
                                            op=mybir.AluOpType.add)

                    # reduce over the other axis, then softmax over this one
                    red = spool.tile([128, 16], F32, tag="red")
                    if reduce_axis == "j":
                        nc.vector.tensor_reduce(
                            out=red[:], in_=s[:], axis=mybir.AxisListType.X,
                            op=mybir.AluOpType.max)
                    else:
                        nc.vector.tensor_reduce(
                            out=red[:], in_=s[:].rearrange("p i j -> p j i"),
                            axis=mybir.AxisListType.X, op=mybir.AluOpType.max)
                    nm1 = spool.tile([128, 1], F32, tag="nm1")
                    nc.vector.tensor_reduce(out=nm1[:], in_=red[:],
                                            axis=mybir.AxisListType.X,
                                            op=mybir.AluOpType.max, negate=True)
                    ez = gpool.tile([128, 16], F32, tag=f"ez{reduce_axis}{mt}")
                    ssum = spool.tile([128, 1], F32, tag="ssum")
                    nc.scalar.activation(ez[:], red[:],
                                         mybir.ActivationFunctionType.Exp,
                                         bias=nm1[:], scale=1.0,
                                         accum_out=ssum[:])
                    rs = gpool.tile([128, 1], F32, tag=f"rs{reduce_axis}{mt}")
                    nc.vector.reciprocal(rs[:], ssum[:])
                    # keep weights unnormalized; the output rows are scaled
                    # by rs after the weighted-sum matmul
                    weights.append((ez, rs))
                return weights

            hw = phase_b(ohHh, ohTh, 0, 1, "j")   # h-order: softmax over i
            tw = phase_b(ohHt, ohTt, 2, 3, "i")   # t-order: softmax over j

            # ---- phase C: out rows = G.T @ band ----
            def phase_c(weights, ent_off, vcol, band, nb, out_dram):
                wT = gpool.tile([16, PPC], F32R, tag="wT")
                for mt in range(MT):
                    tp = pgpool.tile([16, 128], F32, space="PSUM", tag="sm")
                    nc.tensor.transpose(tp[:], weights[mt][0][:], ident[:])
                    nc.vector.tensor_copy(wT[:, mt * 128:(mt + 1) * 128], tp[:])
                # replicate wT 8x along partitions with one K=16 matmul
                wrep = pgpool.tile([128, PPC], F32, space="PSUM", tag="sm")
                nc.tensor.matmul(wrep[:], lhsT=repm_sb[:], rhs=wT[:],
                                 start=True, stop=True)
                gts = []
                for kt in range(nb):
                    gt = gpool.tile([128, PPC], DTYPE_MM, tag=f"gt{kt}")
                    nc.vector.scalar_tensor_tensor(
                        out=gt[:],
                        in0=vrep[:, vcol * PPC:(vcol + 1) * PPC],
                        scalar=entcols_sb[:, ent_off + kt:ent_off + kt + 1],
                        in1=wrep[:],
                        op0=mybir.AluOpType.is_equal,
                        op1=mybir.AluOpType.mult)
                    gts.append(gt)
                for mt in range(MT):
                    ps = ppool.tile([128, H], F32, space="PSUM", tag="proj")
                    for half, w0, w1 in ((0, 0, 512), (1, 512, 768)):
                        for kt in range(nb):
                            nc.tensor.matmul(
                                ps[:, w0:w1],
                                lhsT=gts[kt][:, mt * 128:(mt + 1) * 128],
                                rhs=band[kt][:, w0:w1],
                                start=(kt == 0), stop=(kt == nb - 1))
                    o = wpool.tile([128, H], F32, tag="o")
                    if mt % 2 == 0:
                        nc.vector.tensor_scalar_mul(o[:], ps[:], weights[mt][1][:])
                        nc.sync.dma_start(
                            out_dram.ap()[mt * 128:(mt + 1) * 128, :], o[:])
                    else:
                        nc.scalar.activation(
                            o[:], ps[:], mybir.ActivationFunctionType.Copy,
                            scale=weights[mt][1][:])
                        nc.gpsimd.dma_start(
                            out_dram.ap()[mt * 128:(mt + 1) * 128, :], o[:])

            phase_c(hw, 0, 4, hfb, NBH, reh)
            phase_c(tw, NBH, 5, tfb, NBT, ret)

    _split_multi_waits(nc)
    return nc


_CACHE = {}


def kernel(**inputs):
    meta, shared, per_core, post = _prep(inputs)
    key = tuple(sorted(meta.items()))
    if key not in _CACHE:
        _CACHE[key] = _build(meta)
    nc = _CACHE[key]

    in_maps = []
    for k in range(NC):
        m = dict(per_core[k])
        m.update(shared)
        m = {name: np.ascontiguousarray(v, np.float32) for name, v in m.items()}
        in_maps.append({
            "xt_h": m["xt_h"], "xt_t": m["xt_t"], "wts": m["wts"],
            "wvec": m["wvec"], "iota16": m["iota16"], "iotaP": m["iotaP"],
            "vals": m["vals"], "entcols": m["entcols"],
            "nums": m["nums"], "identity": m["identity"], "repm": m["repm"],
        })

    res = run_bass_kernel_spmd(nc, in_maps, list(range(NC)))

    start_re = np.empty((N, H), np.float32)
    end_re = np.empty((N, H), np.float32)
    h_order, t_order = post["h_order"], post["t_order"]
    for k in range(NC):
        start_re[h_order[k * PPC:(k + 1) * PPC]] = res.results[k]["reh"]
        end_re[t_order[k * PPC:(k + 1) * PPC]] = res.results[k]["ret"]

    entity = np.asarray(inputs["entity_embed"], np.float32)
    b_ind = np.asarray(inputs["b_ind"]).astype(np.int64)
    h_ind = np.asarray(inputs["h_ind"]).astype(np.int64)
    t_ind = np.asarray(inputs["t_ind"]).astype(np.int64)
    head_embed = np.concatenate([entity[b_ind, h_ind], start_re], axis=-1)
    tail_embed = np.concatenate([entity[b_ind, t_ind], end_re], axis=-1)
    return head_embed, tail_embed


# revision 35
# speedup vs baseline: 2.0442x; 2.0442x over previous
"""Trainium2 Bass kernel for nn_DualAttention (sparse_attention).

Algorithm notes
---------------
The reference gathers per-pair mention blocks hfo/tfo = mention_embed[b, h/t]
([N,16,768]) and projects them per pair. But the projections depend only on
the (batch, entity) index, of which there are just B*E = 168, so we compute
relu(X @ W.T) per *entity* (24x less matmul work), then combine per pair:

  s[n,i,j] = hc[eh][i] + qv[et][j] + tq[et][i] * hf16[eh][i,j]   (+ masks)
  h_weight = softmax_i(max_j s);  start_re = h_weight @ hf[eh]
  t_weight = softmax_j(max_i s);  end_re   = t_weight @ tf[et]

Distribution over 8 cores: pairs are sorted by head entity (and separately by
tail entity); core k takes sorted block k of 512 pairs, so its pairs touch a
narrow contiguous band of entities. Each core projects only its band rows
(host passes the pre-transposed mention rows for the band), computes the
small per-entity tables (hc, hf16 / qv, tq) for its band, and an AllGather
shares those tables (tiny) with all cores. Per-pair gathers from the tables
are one-hot matmuls; the final weighted sums are banded matmuls
G.T @ hf_band where G is built on-device from the softmax weights and a
host-provided band mask. The entity_embed half of the outputs is a pure
input gather and is assembled on the host.

Matmuls run in float32r (full-rate fp32 with reduced mantissa, ~1e-4 rel
err); set DTYPE_MM = "f32" for exact-rate-limited fp32.
"""

import numpy as np

import concourse.bass as bass
import concourse.mybir as mybir
import concourse.tile as tile
from concourse.bass_utils import run_bass_kernel_spmd

# problem constants
H = 768
B, E, M = 4, 42, 16
NENT = B * E            # 168
N = 4096
NC = 8
PPC = N // NC           # 512 pairs per core
MT = PPC // 128         # 4 m-tiles of pairs per core
KT = H // 128           # 6 k-tiles over hidden dim
NEG = -1e9

F32 = mybir.dt.float32
F32R = mybir.dt.float32r
DTYPE_MM = F32R         # matmul dtype for the heavy matmuls


def _split_multi_waits(nc, max_waits=1):
    """walrus codegen in this container rejects >1 sync wait per instruction.

    Move extra waits onto pure-wait EventSemaphore instructions inserted just
    before, on the same engine (engine queues are serial, so ordering and
    semantics are preserved)."""
    for fn in nc.m.functions:
        for bb in fn.blocks:
            new = []
            changed = False
            for ins in bb.instructions:
                si = ins.sync_info
                if si is not None and si.on_wait and len(si.on_wait) > max_waits:
                    waits = list(si.on_wait)
                    for i, w in enumerate(waits[:-max_waits]):
                        ev = mybir.InstEventSemaphore(
                            name=f"{ins.name}-xw{i}", engine=ins.engine
                        )
                        ev.sync_info = mybir.SyncInfo(on_wait=[w], on_update=[])
                        ev.debug = ins.debug
                        new.append(ev)
                    si.on_wait = waits[-max_waits:]
                    changed = True
                new.append(ins)
            if changed:
                bb.instructions = new


def _band(ent_sorted):
    """Aligned 128-row band of mention rows covering the given entities."""
    lo_row = 16 * int(ent_sorted.min())
    hi_row = 16 * int(ent_sorted.max()) + 16
    lo_tile = lo_row // 128
    nb = (hi_row - 128 * lo_tile + 127) // 128
    return lo_tile, nb


def _prep(inputs):
    """Host-side sharding: indices, sort orders, bands, one-hots, masks."""
    f32 = np.float32
    mention = np.ascontiguousarray(inputs["mention_embed"], dtype=f32)
    mention_flat = mention.reshape(NENT * M, H)          # row 16*e + i
    b_ind = np.asarray(inputs["b_ind"]).astype(np.int64)
    h_ind = np.asarray(inputs["h_ind"]).astype(np.int64)
    t_ind = np.asarray(inputs["t_ind"]).astype(np.int64)
    mention_num = np.asarray(inputs["mention_num"]).astype(np.int64)

    eh = (b_ind * E + h_ind).astype(np.int64)
    et = (b_ind * E + t_ind).astype(np.int64)
    mnum_flat = mention_num.reshape(NENT)

    h_order = np.argsort(eh, kind="stable")
    t_order = np.argsort(et, kind="stable")

    lo_h, nb_h, lo_t, nb_t = [], [], [], []
    for k in range(NC):
        lo, nb = _band(eh[h_order[k * PPC:(k + 1) * PPC]])
        lo_h.append(lo); nb_h.append(nb)
        lo, nb = _band(et[t_order[k * PPC:(k + 1) * PPC]])
        lo_t.append(lo); nb_t.append(nb)
    NBH = max(nb_h)
    NBT = max(nb_t)

    # table slot count per core: power-of-two-ish divisor of 128
    def slots_for(nb):
        need = nb * 8
        for s in (16, 32, 64, 128):
            if need <= s:
                return s
        raise ValueError(f"band too wide: {nb} tiles")
    SLH = slots_for(NBH)
    SLT = slots_for(NBT)
    TH = NC * SLH // 128        # Htable k-tiles
    TT = NC * SLT // 128

    # owner core + table row for each entity (first band containing it)
    def table_rows(lo_list, nb, slots):
        rows = np.full(NENT, -1, np.int64)
        for k in reversed(range(NC)):
            base_ent = 8 * lo_list[k]
            ents = np.arange(base_ent, min(base_ent + nb * 8, NENT))
            rows[ents] = k * slots + (ents - base_ent)
        return rows
    hrow = table_rows(lo_h, NBH, SLH)
    trow = table_rows(lo_t, NBT, SLT)

    iota16 = np.tile(np.arange(16, dtype=f32)[None, :], (128, 1))

    per_core = []
    for k in range(NC):
        ph = h_order[k * PPC:(k + 1) * PPC]
        pt = t_order[k * PPC:(k + 1) * PPC]
        ehh, eth = eh[ph], et[ph]      # entity ids for h-ordered pairs
        eht, ett = eh[pt], et[pt]      # ... for t-ordered pairs

        # band mention rows, transposed, zero-padded
        def xt_for(lo, nb):
            rows = np.zeros((nb * 128, H), f32)
            g0 = 128 * lo
            g1 = min(g0 + nb * 128, NENT * M)
            rows[: g1 - g0] = mention_flat[g0:g1]
            return np.ascontiguousarray(rows.T)          # [768, nb*128]
        xt_h = xt_for(lo_h[k], NBH)
        xt_t = xt_for(lo_t[k], NBT)

        # value vectors for device-built one-hots and band masks:
        # [hrow(ehh) | trow(eth) | hrow(eht) | trow(ett) | eh(h-order) | et(t-order)]
        vals = np.concatenate([
            hrow[ehh], trow[eth], hrow[eht], trow[ett], ehh, ett,
        ]).astype(f32)[None, :]                           # [1, 6*512]

        # entcols[p, c]: global entity id of band row 128*c + p (head cols
        # first, then tail cols)
        pp = np.arange(128)
        entc = [((128 * lo_h[k] + 128 * c + pp) // 16).astype(f32)
                for c in range(NBH)]
        entc += [((128 * lo_t[k] + 128 * c + pp) // 16).astype(f32)
                 for c in range(NBT)]
        entcols = np.stack(entc, axis=1)                  # [128, NBH+NBT]

        # per-pair mention counts as [128, MT] per-partition scalars
        def nums(vals):
            return np.ascontiguousarray(
                vals.astype(f32).reshape(MT, 128).T)     # [128, MT]
        numcols = np.concatenate(
            [nums(mnum_flat[ehh]), nums(mnum_flat[eth]),
             nums(mnum_flat[eht]), nums(mnum_flat[ett])], axis=1)  # [128,16]

        per_core.append(dict(
            xt_h=xt_h, xt_t=xt_t, vals=vals, entcols=entcols,
            nums=numcols,
        ))

    wts = np.ascontiguousarray(np.concatenate(
        [np.asarray(inputs["W_head"], dtype=f32).T,
         np.asarray(inputs["W_tail"], dtype=f32).T], axis=1))     # [768, 1536]
    wvec = np.concatenate([
        np.tile(np.asarray(inputs["w_c"], f32)[None, :], (128, 1)),
        np.tile(np.asarray(inputs["w_q"], f32)[None, :], (128, 1)),
        np.tile(np.asarray(inputs["w_cq"], f32)[None, :], (128, 1)),
    ], axis=1)                                                    # [128, 2304]

    meta = dict(NBH=NBH, NBT=NBT, SLH=SLH, SLT=SLT, TH=TH, TT=TT)
    TMX = max(TH, TT)
    iotaP = np.stack([(128 * c + np.arange(128)).astype(f32)
                      for c in range(TMX)], axis=1)       # [128, TMX]
    repm = (np.arange(16)[:, None] == (np.arange(128) % 16)[None, :]).astype(f32)
    shared = dict(wts=wts, wvec=wvec, iota16=iota16, iotaP=iotaP,
                  identity=np.eye(128, dtype=f32), repm=repm)
    post = dict(h_order=h_order, t_order=t_order, eh=eh, et=et)
    return meta, shared, per_core, post


def _build(meta, sim_single=False):
    NBH, NBT = meta["NBH"], meta["NBT"]
    SLH, SLT = meta["SLH"], meta["SLT"]
    TH, TT = meta["TH"], meta["TT"]
    # two allgather shards: head = [hc | hf16], tail = [qv | tq] — split so
    # the head collective can start while tail projections still run
    sec_hc = 0
    sec_hf16 = SLH * 16
    SHARD_H = SLH * 16 * 17
    sec_qv = 0
    sec_tq = SLT * 16
    SHARD_T = SLT * 32

    nc = bass.Bass("TRN2", num_devices=(1 if sim_single else NC))
    xt_h = nc.dram_tensor("xt_h", [H, NBH * 128], F32R, kind="ExternalInput")
    xt_t = nc.dram_tensor("xt_t", [H, NBT * 128], F32R, kind="ExternalInput")
    wts = nc.dram_tensor("wts", [H, 2 * H], F32R, kind="ExternalInput")
    wvec = nc.dram_tensor("wvec", [128, 3 * H], F32, kind="ExternalInput")
    iota16 = nc.dram_tensor("iota16", [128, 16], F32, kind="ExternalInput")
    TMX = max(TH, TT)
    vals = nc.dram_tensor("vals", [1, 6 * PPC], F32, kind="ExternalInput")
    entcols = nc.dram_tensor("entcols", [128, NBH + NBT], F32, kind="ExternalInput")
    iotaP = nc.dram_tensor("iotaP", [128, TMX], F32, kind="ExternalInput")
    nums = nc.dram_tensor("nums", [128, 4 * MT], F32, kind="ExternalInput")
    identity = nc.dram_tensor("identity", [128, 128], F32, kind="ExternalInput")
    repm = nc.dram_tensor("repm", [16, 128], F32, kind="ExternalInput")
    reh = nc.dram_tensor("reh", [PPC, H], F32, kind="ExternalOutput")
    ret = nc.dram_tensor("ret", [PPC, H], F32, kind="ExternalOutput")

    with tile.TileContext(nc, num_cores=NC) as tc:
        with (
            tc.tile_pool(name="const", bufs=1) as cpool,
            tc.tile_pool(name="band", bufs=1) as bpool,
            tc.tile_pool(name="work", bufs=3) as wpool,
            tc.tile_pool(name="keep", bufs=1) as gpool,
            tc.tile_pool(name="small", bufs=4) as spool,
            # "proj" big psum: [128,768] = 2 banks x 2 bufs; "sm" small psum:
            # 1 bank x 4 bufs -> 8 banks total, exactly PSUM capacity
            tc.tile_pool(name="psum", bufs=2, space="PSUM") as ppool,
            tc.tile_pool(name="psg", bufs=4, space="PSUM") as pgpool,
            tc.tile_pool(name="dram", bufs=1, space="DRAM") as dpool,
        ):
            # ---- big matmul operands first, interleaved by k-tile, so the
            # first projections can start as soon as their slices land ----
            wt_r, xth, xtt = [], [], []
            for kt in range(KT):
                w = cpool.tile([128, 2 * H], DTYPE_MM, tag=f"wt{kt}")
                nc.sync.dma_start(w[:, H:], wts.ap()[kt * 128:(kt + 1) * 128, H:])
                tt = bpool.tile([128, NBT * 128], DTYPE_MM, tag=f"xtt{kt}")
                nc.sync.dma_start(tt[:], xt_t.ap()[kt * 128:(kt + 1) * 128, :])
                wt_r.append(w)
                xtt.append(tt)
            for kt in range(KT):
                nc.sync.dma_start(
                    wt_r[kt][:, :H], wts.ap()[kt * 128:(kt + 1) * 128, :H])
                th = bpool.tile([128, NBH * 128], DTYPE_MM, tag=f"xth{kt}")
                nc.sync.dma_start(th[:], xt_h.ap()[kt * 128:(kt + 1) * 128, :])
                xth.append(th)
            wvec_sb = cpool.tile([128, 3 * H], F32)
            nc.sync.dma_start(wvec_sb[:], wvec.ap())
            iota_sb = cpool.tile([128, 16], F32)
            nc.sync.dma_start(iota_sb[:], iota16.ap())
            nums_sb = cpool.tile([128, 4 * MT], F32)
            nc.sync.dma_start(nums_sb[:], nums.ap())
            entcols_sb = cpool.tile([128, NBH + NBT], F32)
            nc.sync.dma_start(entcols_sb[:], entcols.ap())
            iotaP_sb = cpool.tile([128, TMX], F32)
            nc.sync.dma_start(iotaP_sb[:], iotaP.ap())
            vrep = cpool.tile([128, 6 * PPC], F32)
            nc.gpsimd.dma_start(
                vrep[:],
                bass.AP(tensor=vals.ap().tensor, offset=0,
                        ap=[[0, 128], [1, 6 * PPC]]))
            ident = cpool.tile([128, 128], F32)
            nc.gpsimd.dma_start(ident[:], identity.ap())
            repm_sb = cpool.tile([16, 128], F32R)
            nc.gpsimd.dma_start(repm_sb[:], repm.ap())

            # ---- phase A: band projections hf = relu(X @ W_head.T) ----
            def project(xt_tiles, nb, woff, tag):
                out = []
                for mt in range(nb):
                    ps = ppool.tile([128, H], F32, space="PSUM", tag="proj")
                    for half in range(2):
                        sl = slice(woff + half * 512, woff + min(768, (half + 1) * 512))
                        for kt in range(KT):
                            nc.tensor.matmul(
                                ps[:, half * 512: half * 512 + (sl.stop - sl.start)],
                                lhsT=xt_tiles[kt][:, mt * 128:(mt + 1) * 128],
                                rhs=wt_r[kt][:, sl],
                                start=(kt == 0), stop=(kt == KT - 1),
                            )
                    t = bpool.tile([128, H], DTYPE_MM, tag=f"{tag}{mt}")
                    if mt % 2 == 0:
                        nc.scalar.activation(t[:], ps[:],
                                             mybir.ActivationFunctionType.Relu)
                    else:
                        nc.vector.tensor_scalar_max(t[:], ps[:], 0.0)
                    out.append(t)
                return out
            tfb = project(xtt, NBT, H, "tfb")
            hfb = project(xth, NBH, 0, "hfb")

            # ---- phase A2: per-entity smalls + allgather ----
            stage_h = dpool.tile([SHARD_H], F32R)
            stage_t = dpool.tile([SHARD_T], F32R)
            cc_h = dpool.tile(
                [NC * SHARD_H], F32R,
                **({} if sim_single else {"addr_space": "Shared"}))
            cc_t = dpool.tile(
                [NC * SHARD_T], F32R,
                **({} if sim_single else {"addr_space": "Shared"}))
            assert SHARD_H % 128 == 0 and SHARD_T % 128 == 0
            zero_sb = cpool.tile([128, SHARD_H // 128], F32)
            nc.vector.memset(zero_sb, 0.0)
            nc.sync.dma_start(
                stage_h[:].rearrange("(p c) -> p c", p=128),
                zero_sb[:, : SHARD_H // 128].bitcast(F32R),
            )
            nc.sync.dma_start(
                stage_t[:].rearrange("(p c) -> p c", p=128),
                zero_sb[:, : SHARD_T // 128].bitcast(F32R),
            )

            def rowdot(src_tile, wcol, acc_ap, eng=None):
                prod = wpool.tile([128, H], F32, tag="prod")
                (eng or nc.vector).tensor_tensor(
                    out=prod[:], in0=src_tile[:].bitcast(F32),
                    in1=wvec_sb[:, wcol * H:(wcol + 1) * H],
                    op=mybir.AluOpType.mult)
                with nc.allow_low_precision(reason="f32r stage values"):
                    nc.scalar.activation(
                        prod[:], prod[:], mybir.ActivationFunctionType.Copy,
                        accum_out=acc_ap)

            def acc_dma(stage, sec, acc_tile, nb):
                dst = bass.AP(tensor=stage[:].tensor, offset=sec,
                              ap=[[1, 128], [128, nb]])
                nc.sync.dma_start(dst, acc_tile[:, :nb])

            acc_hc = spool.tile([128, NBH], F32R, tag="acc_hc")
            acc_qv = spool.tile([128, NBT], F32R, tag="acc_qv")
            acc_tq = spool.tile([128, NBT], F32R, tag="acc_tq")
            def do_collective(stage, cc):
                if sim_single:
                    sz = stage.shape[0]
                    nc.sync.dma_start(cc[0:sz], stage[:])
                    nc.sync.dma_start(cc[(NC - 1) * sz: NC * sz], stage[:])
                else:
                    nc.gpsimd.collective_compute(
                        "AllGather", mybir.AluOpType.bypass,
                        replica_groups=[list(range(NC))],
                        ins=[stage.opt()], outs=[cc.opt()],
                    )

            for mt in range(NBT):
                rowdot(tfb[mt], 1, acc_qv[:, mt:mt + 1])
                rowdot(tfb[mt], 2, acc_tq[:, mt:mt + 1])
            acc_dma(stage_t, sec_qv, acc_qv, NBT)
            acc_dma(stage_t, sec_tq, acc_tq, NBT)
            do_collective(stage_t, cc_t)

            for mt in range(NBH):
                rowdot(hfb[mt], 0, acc_hc[:, mt:mt + 1])
                nc.sync.dma_start(
                    stage_h[sec_hf16 + mt * 2048: sec_hf16 + (mt + 1) * 2048]
                    .rearrange("(p c) -> p c", p=128),
                    hfb[mt][:, :16],
                )
            acc_dma(stage_h, sec_hc, acc_hc, NBH)
            do_collective(stage_h, cc_h)


            # ---- build gathered tables in SBUF ----
            def tbl_src(cc, shard, sec, per_slot, tt, slots, width):
                cores_per_tile = 128 // slots
                return bass.AP(
                    tensor=cc[:].tensor,
                    offset=tt * cores_per_tile * shard + sec,
                    ap=[[shard, cores_per_tile], [per_slot, slots], [1, width]],
                )
            Htab, Ttab = [], []
            for tt in range(TH):
                t = cpool.tile([128, 272], DTYPE_MM, tag=f"htab{tt}")
                nc.gpsimd.dma_start(
                    t[:, 0:16], tbl_src(cc_h, SHARD_H, sec_hc, 16, tt, SLH, 16))
                nc.gpsimd.dma_start(
                    t[:, 16:272], tbl_src(cc_h, SHARD_H, sec_hf16, 256, tt, SLH, 256))
                Htab.append(t)
            for tt in range(TT):
                t = cpool.tile([128, 32], DTYPE_MM, tag=f"ttab{tt}")
                nc.sync.dma_start(
                    t[:, 0:16], tbl_src(cc_t, SHARD_T, sec_qv, 16, tt, SLT, 16))
                nc.sync.dma_start(
                    t[:, 16:32], tbl_src(cc_t, SHARD_T, sec_tq, 16, tt, SLT, 16))
                Ttab.append(t)

            # ---- device-built one-hots: oh[p, n] = (rowvals[n] == 128*kt+p)
            def build_oh(vcol, ktiles, tag):
                tiles = []
                for kt in range(ktiles):
                    t = bpool.tile([128, PPC], DTYPE_MM, tag=f"{tag}{kt}")
                    nc.gpsimd.tensor_scalar(
                        out=t[:],
                        in0=vrep[:, vcol * PPC:(vcol + 1) * PPC],
                        scalar1=iotaP_sb[:, kt:kt + 1], scalar2=None,
                        op0=mybir.AluOpType.is_equal)
                    tiles.append(t)
                return tiles
            ohHh = build_oh(0, TH, "ohHh")
            ohTh = build_oh(1, TT, "ohTh")
            ohHt = build_oh(2, TH, "ohHt")
            ohTt = build_oh(3, TT, "ohTt")

            # ---- precomputed additive masks (only need nums+iota, so these
            # run at t~0 in the DMA shadow): m[col][p,i] = (i>=num)*NEG ----
            masks = []
            for col in range(4):
                for mt in range(MT):
                    mk = gpool.tile([128, 16], F32, tag=f"msk{col}_{mt}")
                    nc.vector.tensor_scalar(
                        out=mk[:], in0=iota_sb[:],
                        scalar1=nums_sb[:, col * MT + mt: col * MT + mt + 1],
                        scalar2=NEG,
                        op0=mybir.AluOpType.is_ge, op1=mybir.AluOpType.mult)
                    masks.append(mk)

            # ---- phase B: per-pair scores + softmax weights ----
            def phase_b(ohH, ohT, hn_col, tn_col, reduce_axis):
                """Returns list of MT [128,16] weight tiles."""
                weights = []
                for mt in range(MT):
                    gps = pgpool.tile([128, 272], F32, space="PSUM", tag="sm")
                    for kt in range(TH):
                        nc.tensor.matmul(
                            gps[:], lhsT=ohH[kt][:, mt * 128:(mt + 1) * 128],
                            rhs=Htab[kt][:], start=(kt == 0), stop=(kt == TH - 1))
                    Hg = gps

                    tps = pgpool.tile([128, 32], F32, space="PSUM", tag="sm")
                    for kt in range(TT):
                        nc.tensor.matmul(
                            tps[:], lhsT=ohT[kt][:, mt * 128:(mt + 1) * 128],
                            rhs=Ttab[kt][:], start=(kt == 0), stop=(kt == TT - 1))
                    Tg = wpool.tile([128, 32], F32, tag="Tg")
                    nc.vector.tensor_copy(Tg[:], tps[:])

                    # s[p,i,j] = (hc[i]+maski[i]) + (qv[j]+maskj[j]) + tq[i]*F[i,j]
                    mi = masks[hn_col * MT + mt]
                    mj = masks[tn_col * MT + mt]
                    a2 = spool.tile([128, 16], F32, tag="a2")
                    nc.vector.tensor_tensor(out=a2[:], in0=Hg[:, 0:16],
                                            in1=mi[:], op=mybir.AluOpType.add)
                    b2 = spool.tile([128, 16], F32, tag="b2")
                    nc.vector.tensor_tensor(out=b2[:], in0=Tg[:, 0:16],
                                            in1=mj[:], op=mybir.AluOpType.add)
                    s = wpool.tile([128, 16, 16], F32, tag="s")
                    nc.vector.tensor_tensor(
                        out=s[:],
                        in0=Tg[:, 16:32, None].to_broadcast((128, 16, 16)),
                        in1=Hg[:, 16:272].rearrange("p (i j) -> p i j", i=16),
                        op=mybir.AluOpType.mult)
                    u = wpool.tile([128, 16, 16], F32, tag="u")
                    nc.gpsimd.tensor_tensor(
                        out=u[:],
                        in0=a2[:, :, None].to_broadcast((128, 16, 16)),
                        in1=b2[:, None, 0:16].to_broadcast((128, 16, 16)),
                        op=mybir.AluOpType.add)
                    nc.vector.tensor_tensor(out=s[:], in0=s[:], in1=u[:],
                                            op=mybir.AluOpType.add)

                    # reduce over the other axis, then softmax over this one
                    red = spool.tile([128, 16], F32, tag="red")
                    if reduce_axis == "j":
                        nc.vector.tensor_reduce(
                            out=red[:], in_=s[:], axis=mybir.AxisListType.X,
                            op=mybir.AluOpType.max)
                    else:
                        nc.vector.tensor_reduce(
                            out=red[:], in_=s[:].rearrange("p i j -> p j i"),
                            axis=mybir.AxisListType.X, op=mybir.AluOpType.max)
                    nm1 = spool.tile([128, 1], F32, tag="nm1")
                    nc.vector.tensor_reduce(out=nm1[:], in_=red[:],
                                            axis=mybir.AxisListType.X,
                                            op=mybir.AluOpType.max, negate=True)
                    ez = gpool.tile([128, 16], F32, tag=f"ez{reduce_axis}{mt}")
                    ssum = spool.tile([128, 1], F32, tag="ssum")
                    nc.scalar.activation(ez[:], red[:],
                                         mybir.ActivationFunctionType.Exp,
                                         bias=nm1[:], scale=1.0,
                                         accum_out=ssum[:])
                    rs = gpool.tile([128, 1], F32, tag=f"rs{reduce_axis}{mt}")
                    nc.vector.reciprocal(rs[:], ssum[:])
                    # keep weights unnormalized; the output rows are scaled
                    # by rs after the weighted-sum matmul
                    weights.append((ez, rs))
                return weights

            hw = phase_b(ohHh, ohTh, 0, 1, "j")   # h-order: softmax over i
            tw = phase_b(ohHt, ohTt, 2, 3, "i")   # t-order: softmax over j

            # ---- phase C: out rows = G.T @ band ----
            def phase_c(weights, ent_off, vcol, band, nb, out_dram):
                wT = gpool.tile([16, PPC], F32R, tag="wT")
                for mt in range(MT):
                    tp = pgpool.tile([16, 128], F32, space="PSUM", tag="sm")
                    nc.tensor.transpose(tp[:], weights[mt][0][:], ident[:])
                    nc.vector.tensor_copy(wT[:, mt * 128:(mt + 1) * 128], tp[:])
                # replicate wT 8x along partitions with one K=16 matmul
                wrep = pgpool.tile([128, PPC], F32, space="PSUM", tag="sm")
                nc.tensor.matmul(wrep[:], lhsT=repm_sb[:], rhs=wT[:],
                                 start=True, stop=True)
                gts = []
                for kt in range(nb):
                    gt = gpool.tile([128, PPC], DTYPE_MM, tag=f"gt{kt}")
                    nc.vector.scalar_tensor_tensor(
                        out=gt[:],
                        in0=vrep[:, vcol * PPC:(vcol + 1) * PPC],
                        scalar=entcols_sb[:, ent_off + kt:ent_off + kt + 1],
                        in1=wrep[:],
                        op0=mybir.AluOpType.is_equal,
                        op1=mybir.AluOpType.mult)
                    gts.append(gt)
                for mt in range(MT):
                    ps = ppool.tile([128, H], F32, space="PSUM", tag="proj")
                    for half, w0, w1 in ((0, 0, 512), (1, 512, 768)):
                        for kt in range(nb):
                            nc.tensor.matmul(
                                ps[:, w0:w1],
                                lhsT=gts[kt][:, mt * 128:(mt + 1) * 128],
                                rhs=band[kt][:, w0:w1],
                                start=(kt == 0), stop=(kt == nb - 1))
                    o = wpool.tile([128, H], F32, tag="o")
                    if mt % 2 == 0:
                        nc.vector.tensor_scalar_mul(o[:], ps[:], weights[mt][1][:])
                        nc.sync.dma_start(
                            out_dram.ap()[mt * 128:(mt + 1) * 128, :], o[:])
                    else:
                        nc.scalar.activation(
                            o[:], ps[:], mybir.ActivationFunctionType.Copy,
                            scale=weights[mt][1][:])
                        nc.gpsimd.dma_start(
                            out_dram.ap()[mt * 128:(mt + 1) * 128, :], o[:])

            phase_c(hw, 0, 4, hfb, NBH, reh)
            phase_c(tw, NBH, 5, tfb, NBT, ret)

    _split_multi_waits(nc)
    return nc


_CACHE = {}


def kernel(**inputs):
    meta, shared, per_core, post = _prep(inputs)
    key = tuple(sorted(meta.items()))
    if key not in _CACHE:
        _CACHE[key] = _build(meta)
    nc = _CACHE[key]

    in_maps = []
    for k in range(NC):
        m = dict(per_core[k])
        m.update(shared)
        m = {name: np.ascontiguousarray(v, np.float32) for name, v in m.items()}
        in_maps.append({
            "xt_h": m["xt_h"], "xt_t": m["xt_t"], "wts": m["wts"],
            "wvec": m["wvec"], "iota16": m["iota16"], "iotaP": m["iotaP"],
            "vals": m["vals"], "entcols": m["entcols"],
            "nums": m["nums"], "identity": m["identity"], "repm": m["repm"],
        })

    res = run_bass_kernel_spmd(nc, in_maps, list(range(NC)))

    start_re = np.empty((N, H), np.float32)
    end_re = np.empty((N, H), np.float32)
    h_order, t_order = post["h_order"], post["t_order"]
    for k in range(NC):
        start_re[h_order[k * PPC:(k + 1) * PPC]] = res.results[k]["reh"]
        end_re[t_order[k * PPC:(k + 1) * PPC]] = res.results[k]["ret"]

    entity = np.asarray(inputs["entity_embed"], np.float32)
    b_ind = np.asarray(inputs["b_ind"]).astype(np.int64)
    h_ind = np.asarray(inputs["h_ind"]).astype(np.int64)
    t_ind = np.asarray(inputs["t_ind"]).astype(np.int64)
    head_embed = np.concatenate([entity[b_ind, h_ind], start_re], axis=-1)
    tail_embed = np.concatenate([entity[b_ind, t_ind], end_re], axis=-1)
    return head_embed, tail_embed
